# revision 3
# baseline (speedup 1.0000x reference)
"""Trainium2 Bass kernel for nn_Attention_Encode (B=4, N=2048, DIM=1024, H=16, DH=64).

Sharding: 16 heads -> 8 cores x 2 heads (tensor parallel). Each core computes
  ztu_g = W_g @ ZT^T          (its 128 output channels = 2 heads)
  attention per (batch, head) with Q=K=V=ztu
  partial_out = ssa_g @ W_g   (row-sharded output projection)
Host sums the 8 partials (the all-reduce step of a row-sharded projection).

v2b: Q=K=V symmetry is used to compute scores QUERY-TILE-major: the QK
stationary is a 128-query tile (K=64 contraction) and the moving operand
streams keys, producing sc[q-tile, keys].  Because the score matrix is
symmetric, sc(t)[q, k] IS the [keys-in-tile-t, queries=k] panel the AV
matmul wants as its moving operand - no transposes anywhere.  The two
heads' K=64 matmuls run concurrently via PE row tiling (head A d-dims on
SBUF partitions 0:63, head B on 64:127, emission interleaved so adjacent
matmuls target different row groups).  The AV stationary [V | ones]
still yields the softmax denominator for free.
"""
import sys

for _p in ('/opt/trn_rl_repo',):
    if _p not in sys.path:
        sys.path.insert(0, _p)

from contextlib import ExitStack

import numpy as np
import ml_dtypes

import concourse.bacc as bacc
import concourse.mybir as mybir
import concourse.tile as tile
from concourse.bass_utils import run_bass_kernel_spmd
from concourse.masks import make_identity

B, N, C = 4, 2048, 1024          # batch, seq, model dim
KP, DH, HPER = 128, 64, 2        # per-core channels, head dim, heads per core
NQB = 512                        # av/norm/proj2 query block
NKT = 128                        # q-tile (=key tile) size
NTB = N // NKT                   # 16 tiles per batch
NTILES = B * NTB                 # 64 n-tiles total
SCALE = DH ** -0.5               # 0.125
BF = mybir.dt.bfloat16
F32 = mybir.dt.float32
F32R = mybir.dt.float32r

_CACHE = {}


def _build_kernel():
    nc = bacc.Bacc("TRN2", target_bir_lowering=False, debug=False)
    ztt = nc.dram_tensor("ztt", [B, C, N], BF, kind="ExternalInput").ap()
    wgt = nc.dram_tensor("wgt", [C, KP], BF, kind="ExternalInput").ap()   # W_g^T
    wg = nc.dram_tensor("wg", [KP, C], BF, kind="ExternalInput").ap()     # W_g
    out = nc.dram_tensor("out", [B * N, C], BF, kind="ExternalOutput").ap()

    with tile.TileContext(nc) as tc, ExitStack() as ctx:
        _body(ctx, tc, ztt, wgt, wg, out)
    nc.compile()
    return nc


def _body(ctx, tc, ztt, wgt, wg, out):
    nc = tc.nc
    singles = ctx.enter_context(tc.tile_pool(name="singles", bufs=1))
    zin_pool = ctx.enter_context(tc.tile_pool(name="zin", bufs=16))
    sc_pool = ctx.enter_context(tc.tile_pool(name="sc", bufs=2, space="PSUM"))
    av_pool = ctx.enter_context(tc.tile_pool(name="av", bufs=4, space="PSUM"))
    ex_pool = ctx.enter_context(tc.tile_pool(name="ex", bufs=8))
    sn_pool = ctx.enter_context(tc.tile_pool(name="sn", bufs=4))
    rc_pool = ctx.enter_context(tc.tile_pool(name="rc", bufs=4))

    # ---- persistent SBUF ----
    wgt_sb = singles.tile([128, 8, KP], BF)            # [c-in-tile, ci, k]
    nc.sync.dma_start(out=wgt_sb, in_=wgt.rearrange("(ci p) k -> p ci k", p=128))
    wg_sb = singles.tile([KP, C], BF)
    nc.sync.dma_start(out=wg_sb, in_=wg)
    ident = singles.tile([128, 128], BF)
    make_identity(nc, ident)
    self_f = singles.tile([128, 128], F32)
    nc.vector.memset(self_f, 0.0)
    nc.vector.memset(self_f[0:1, 0:64], 1.0)
    nc.vector.memset(self_f[32:33, 64:128], 1.0)
    sel = singles.tile([128, 128], F32R)               # den -> per-head row broadcast
    nc.vector.tensor_copy(out=sel, in_=self_f)
    dn = singles.tile([128, NQB], F32R)                # dens: head A row 0, head B row 32
    nc.vector.memset(dn[:].bitcast(F32), 0.0)
    # ztu^T packed: SBUF partitions 0:64 = head A d-dims, 64:128 = head B.
    ztuT = singles.tile([128, B * N], BF)
    # v-natural per head, padded to M=128: cols [v(64) | ones(1) | 0...]
    ztuN = singles.tile([128, NTILES, 2 * NKT], BF)    # [n-in-tile, nt, head*128+c]
    nc.gpsimd.memset(ztuN, 0.0)
    nc.gpsimd.memset(ztuN[:, :, DH:DH + 1], 1.0)
    nc.gpsimd.memset(ztuN[:, :, NKT + DH:NKT + DH + 1], 1.0)

    # ---- phase 1: proj1 (ztuT = W_g @ ZT^T) + phase 1.5: transposes (ztuN) ----
    def load_zin(b):
        zin = []
        for ci in range(8):
            z = zin_pool.tile([128, N], BF, tag="zin", name=f"zin{ci}")
            for jn in range(N // NQB):
                nc.sync.dma_start(
                    out=z[:, jn * NQB:(jn + 1) * NQB],
                    in_=ztt[b, ci * 128:(ci + 1) * 128, jn * NQB:(jn + 1) * NQB])
            zin.append(z)
        return zin

    def proj1_chunk(b, zin, jn):
        p1 = sc_pool.tile([128, 2 * NQB], F32, tag="sc", name="p1")
        p1v = p1[:, 0:NQB]
        for ci in range(8):
            nc.tensor.matmul(
                p1v, lhsT=wgt_sb[:, ci, :],
                rhs=zin[ci][:, jn * NQB:(jn + 1) * NQB],
                start=(ci == 0), stop=(ci == 7),
            )
        nc.vector.tensor_copy(
            out=ztuT[:, b * N + jn * NQB: b * N + (jn + 1) * NQB],
            in_=p1v)

    def transpose_chunk(b, jn):
        # One transpose per n-tile: ztuT rows 0:64 / 64:128 are heads A / B,
        # so pt cols 0:64 / 64:128 are the per-head v-naturals.
        for ntl in range(4 * jn, 4 * jn + 4):
            nt = b * NTB + ntl
            pt = sc_pool.tile([128, NQB], BF, tag="sc", name="pt")
            nc.tensor.transpose(
                pt[:, 0:128],
                ztuT[:, nt * NKT:(nt + 1) * NKT],
                ident,
            )
            for hh in range(HPER):
                nc.vector.tensor_copy(
                    out=ztuN[:, nt, hh * NKT: hh * NKT + DH],
                    in_=pt[:, hh * DH: hh * DH + DH])

    # ---- phase 2: attention, query-tile-major scores, key-pair phases ----
    def qk_qtile(b, t, kp):
        # sc[h] = [128 queries of tile t, 1024 keys (kp half)], one per head.
        # lhsT (stationary, K=64) is the q-tile, reused across the 2 moving
        # key blocks; head emission is key-block-major so adjacent matmuls
        # target different PE row groups and overlap.
        scs = [sc_pool.tile([128, 2 * NQB], F32, tag="sc", name=f"sc{h}")
               for h in range(HPER)]
        q0 = b * N + t * NKT
        k0 = b * N + kp * 2 * NQB
        for u in range(2):
            for hh in range(HPER):
                h0 = hh * DH
                nc.tensor.matmul(
                    scs[hh][:, u * NQB:(u + 1) * NQB],
                    lhsT=ztuT[h0:h0 + DH, q0:q0 + NKT],
                    rhs=ztuT[h0:h0 + DH, k0 + u * NQB: k0 + (u + 1) * NQB],
                    start=True, stop=True)
        return scs

    def exp_qtile(scs):
        exs = []
        for hh in range(HPER):
            ex = ex_pool.tile([128, 2 * NQB], BF, tag="ex")
            nc.scalar.activation(
                out=ex, in_=scs[hh],
                func=mybir.ActivationFunctionType.Exp, scale=SCALE)
            exs.append(ex)
        return exs

    def av_step(b, t, avs4, exs):
        # sc(t)[q, k] == [keys-in-tile-t, queries k] by symmetry: accumulate
        # key-tile t's contribution into both query-block accumulators.
        for u in range(2):
            for hh in range(HPER):
                vT = ztuN[:, b * NTB + t, hh * NKT:(hh + 1) * NKT]
                nc.tensor.matmul(avs4[u][hh], lhsT=vT,
                                 rhs=exs[hh][:, u * NQB:(u + 1) * NQB],
                                 start=(t == 0), stop=(t == NTB - 1))

    def finish_norm(b, jq, avs):
        # softmax denominators -> per-head broadcast -> reciprocal -> scale
        nc.vector.tensor_copy(out=dn[0:1, :], in_=avs[0][DH:DH + 1, :])
        nc.vector.tensor_copy(out=dn[32:33, :], in_=avs[1][DH:DH + 1, :])
        bc = sc_pool.tile([128, NQB], F32, tag="sc", name="bc")
        bcv = bc[:, 0:NQB]
        nc.tensor.matmul(bcv, lhsT=sel, rhs=dn, start=True, stop=True)
        rc = rc_pool.tile([128, NQB], F32)
        nc.vector.reciprocal_approx_fast(out=rc, in_=bcv)
        sn = sn_pool.tile([128, NQB], BF)
        nc.vector.tensor_tensor(
            out=sn[0:64, :], in0=avs[0][0:DH, :], in1=rc[0:64, :],
            op=mybir.AluOpType.mult)
        nc.vector.tensor_tensor(
            out=sn[64:128, :], in0=avs[1][0:DH, :], in1=rc[64:128, :],
            op=mybir.AluOpType.mult)
        return sn

    def finish_proj2(b, jq, sn):
        # proj2: out[q, :] += ssa_norm_g @ W_g  (both heads contracted)
        for t in range(NQB // 128):
            for ch in range(2):
                p2 = sc_pool.tile([128, NQB], F32, tag="sc", name="p2")
                p2v = p2[:, 0:512]
                nc.tensor.matmul(
                    p2v, lhsT=sn[:, t * 128:(t + 1) * 128],
                    rhs=wg_sb[:, ch * 512:(ch + 1) * 512],
                    start=True, stop=True)
                p2s = rc_pool.tile([128, 512], BF, tag="p2s")
                nc.vector.tensor_copy(out=p2s, in_=p2v)
                r0 = b * N + jq * NQB + t * 128
                nc.gpsimd.dma_start(
                    out=out[r0:r0 + 128, ch * 512:(ch + 1) * 512], in_=p2s)

    # Deferred norm/proj2 work queue, flushed inside later compute phases so
    # the in-order PE queue never stalls on the DVE normalization chain.
    state = {"norms": [], "p2s": []}

    def flush_norms():
        for (b, jq, avs) in state["norms"]:
            state["p2s"].append((b, jq, finish_norm(b, jq, avs)))
        state["norms"] = []

    def flush_one_proj2():
        if state["p2s"]:
            finish_proj2(*state["p2s"].pop(0))

    def kp_phase(b, kp, filler=None):
        avs4 = [[av_pool.tile([128, NQB], F32, tag="av", name=f"av{u}{h}")
                 for h in range(HPER)] for u in range(2)]
        flush_norms()
        prev = None
        for t in range(NTB):
            scs = qk_qtile(b, t, kp)
            exs = exp_qtile(scs)
            if prev is not None:
                av_step(b, prev[0], avs4, prev[1])
            prev = (t, exs)
            if t == 7:
                flush_one_proj2()
                if filler is not None:
                    filler(0)
            elif t == 11:
                flush_one_proj2()
            elif t == 15:
                if filler is not None:
                    filler(1)
        av_step(b, prev[0], avs4, prev[1])
        for u in range(2):
            state["norms"].append((b, 2 * kp + u, avs4[u]))

    def attention_batch(b, filler=None):
        for kp in range(2):
            fl = None
            if filler is not None:
                fl = lambda i, kp=kp: filler(2 * kp + i)
            kp_phase(b, kp, fl)

    # PE warm-up spin: ~6us of dependency-free matmuls so the HAM clock gate
    # is already at 8/8 when the first DMA-gated proj1 matmul lands.
    warm = sc_pool.tile([128, NQB], F32, tag="sc", name="warm")
    for _ in range(256):
        nc.tensor.matmul(warm[:, 0:32], lhsT=ident, rhs=ident[:, 0:32],
                         start=True, stop=True)
    del warm

    zs = {0: load_zin(0)}
    for b in range(B):
        if b + 1 < B:
            zs[b + 1] = load_zin(b + 1)
        if b == 0:
            for jn in range(N // NQB):
                proj1_chunk(0, zs[0], jn)
                transpose_chunk(0, jn)
            zs.pop(0)
        else:
            zin = zs.pop(b)

            def filler(jq, b=b, zin=zin):
                proj1_chunk(b, zin, jq)
                transpose_chunk(b, jq)

            attention_batch(b - 1, filler)
    attention_batch(B - 1)
    flush_norms()
    while state["p2s"]:
        flush_one_proj2()


def _get_nc():
    if "nc" not in _CACHE:
        _CACHE["nc"] = _build_kernel()
    return _CACHE["nc"]


def make_in_maps(ZT, W):
    ZT = np.asarray(ZT, dtype=np.float32)
    W = np.asarray(W, dtype=np.float32)
    ztt = np.ascontiguousarray(ZT.transpose(0, 2, 1)).astype(ml_dtypes.bfloat16)
    in_maps = []
    for c in range(8):
        wgf = W[c * KP:(c + 1) * KP, :]
        in_maps.append({
            "ztt": ztt,
            "wgt": np.ascontiguousarray(wgf.T).astype(ml_dtypes.bfloat16),
            "wg": np.ascontiguousarray(wgf).astype(ml_dtypes.bfloat16),
        })
    return in_maps


def kernel(ZT: np.ndarray, W: np.ndarray) -> np.ndarray:
    nc = _get_nc()
    res = run_bass_kernel_spmd(nc, make_in_maps(ZT, W), core_ids=list(range(8)))
    acc = np.zeros((B * N, C), dtype=np.float32)
    for r in res.results:
        acc += r["out"].astype(np.float32)
    return acc.reshape(B, N, C)


if __name__ == "__main__":
    rng = np.random.default_rng(0)
    zt = rng.standard_normal((B, N, C), dtype=np.float32)
    w = rng.standard_normal((KP * 8, C), dtype=np.float32) * C ** -0.5
    o = kernel(zt, w)
    print("out", o.shape, o.dtype, float(np.abs(o).mean()))
